# revision 14
# baseline (speedup 1.0000x reference)
"""ClassSR MoE kernel for 8 Trainium2 NeuronCores.

Strategy (two-phase, routed):
  Phase 1: classifier kernel, data-parallel (4 samples/core) -> logits [32,3].
           Host argmax -> per-sample flag (which SR net).
  Phase 2: every sample runs ONE generic SR net (nf=64, nb=3 shapes) whose
           weights are its flagged net's weights zero-padded to the generic
           shapes (zero channels stay zero through lrelu; zero residual
           blocks act as identity).  Data-parallel, 4 samples/core.

Convs are per-tap matmuls (channels on partitions, spatial in free dim),
accumulated in PSUM, fp32r operands (1 cyc/row).  lrelu on ScalarE (Prelu,
alpha=0.1) fused with the PSUM->SBUF evacuation; the 2x pixel-shuffles are
fused into the activation writes via host-permuted conv output channels.
The 128^2 and 256^2 tails are strip-pipelined through SBUF ring buffers.
"""

import math
import numpy as np

import concourse.bass as bass
import concourse.bacc as bacc
import concourse.mybir as mybir
import concourse.tile as tile
from concourse.bass_utils import run_bass_kernel_spmd

F32 = mybir.dt.float32
F32R = mybir.dt.float32r
PRELU = mybir.ActivationFunctionType.Prelu
IDENT = mybir.ActivationFunctionType.Identity
COPY = mybir.ActivationFunctionType.Copy
ADD = mybir.AluOpType.add

N_CORES = 8
B = 32
SPC = B // N_CORES          # samples per core
NF = 64                     # generic feature channels
NB = 3                      # generic residual blocks
H = W = 64
TAPS = [(dy, dx) for dy in range(3) for dx in range(3)]
ALPHA = 0.1

# ---------------- weight blob layout (columns of a [64, WCOLS] blob) -------
CF_OFF = 0                       # 9 taps x [3, 64]
BLK_OFF = CF_OFF + 9 * 64        # 6 convs x 9 taps x [64, 64]
DEB_OFF = BLK_OFF + 54 * 64      # 9 taps x [64, 3]
UP1_OFF = DEB_OFF + 9 * 3        # 9 taps x 2 mtiles x [64, 128]
UP2_OFF = UP1_OFF + 9 * 256
LAST_OFF = UP2_OFF + 9 * 256     # 9 taps x [64, 3]
WCOLS = LAST_OFF + 9 * 3

# bias blob [128, NBCOL] columns
BC_CF = 0
BC_BLK = 1          # 6 cols (block b conv k -> 1 + 2b + k)
BC_DEB = 7
BC_UP1 = 8          # 2 cols (m-tiles, 128 rows each)
BC_UP2 = 10
BC_LAST = 12
NBCOL = 13


def wcol_cf(t, m):
    return CF_OFF + t * 64


def wcol_blk(conv_idx):
    def f(t, m):
        return BLK_OFF + (conv_idx * 9 + t) * 64
    return f


def wcol_deb(t, m):
    return DEB_OFF + t * 3


def wcol_up(base):
    def f(t, m):
        return base + t * 256 + m * 128
    return f


def wcol_last(t, m):
    return LAST_OFF + t * 3


# =================== host-side weight packing ==============================

def _np(a):
    return np.asarray(a, dtype=np.float32)


def pack_generic_net(p, nf, nb):
    """Pack one net's params (true size nf, nb) into generic (64, 3) blobs."""
    wb = np.zeros((64, WCOLS), np.float32)
    bb = np.zeros((128, NBCOL), np.float32)

    def put_conv(off, w, b, cin_true, cout_true, bias_col):
        w = _np(w)                     # [O, I, 3, 3]
        for t, (dy, dx) in enumerate(TAPS):
            c0 = off(t, 0)
            wb[:cin_true, c0:c0 + cout_true] = w[:, :, dy, dx].T
        bb[:cout_true, bias_col] = _np(b)

    put_conv(wcol_cf, p['cf_w'], p['cf_b'], 3, nf, BC_CF)
    for bi in range(nb):
        w1, b1, w2, b2 = p['blocks'][bi]
        put_conv(wcol_blk(2 * bi), w1, b1, nf, nf, BC_BLK + 2 * bi)
        put_conv(wcol_blk(2 * bi + 1), w2, b2, nf, nf, BC_BLK + 2 * bi + 1)
    put_conv(wcol_deb, p['deb_w'], p['deb_b'], nf, 3, BC_DEB)

    # up convs: output channels permuted for fused pixel-shuffle.
    # generic col (m-tile m, q): group g=q//64, c=q%64, ab=2m+g,
    # original channel = 4c+ab (zero-padded if c >= nf).
    for base, bcol, wk, bk in ((UP1_OFF, BC_UP1, 'up1_w', 'up1_b'),
                               (UP2_OFF, BC_UP2, 'up2_w', 'up2_b')):
        w = _np(p[wk])                 # [4nf, I, 3, 3]
        bv = _np(p[bk])
        cin_true = w.shape[1]
        for m in range(2):
            for q in range(128):
                g, c = q // 64, q % 64
                if c >= nf:
                    continue
                orig = 4 * c + 2 * m + g
                for t, (dy, dx) in enumerate(TAPS):
                    col = base + t * 256 + m * 128 + q
                    wb[:cin_true, col] = w[orig, :, dy, dx]
                bb[q, bcol + m] = bv[orig]
    put_conv(wcol_last, p['last_w'], p['last_b'], nf, 3, BC_LAST)
    return wb, bb


CLS_C0 = 0            # 16 taps x [3, 128]
CLS_C1 = 16 * 128     # 3 x [128, 128]
CLS_C4 = CLS_C1 + 3 * 128
CLS_FW = CLS_C4 + 32  # [33, 3] (row 32 = fb)
CLS_COLS = CLS_FW + 3


def pack_classifier(cls):
    wb = np.zeros((128, CLS_COLS), np.float32)
    bb = np.zeros((128, 5), np.float32)
    c0 = _np(cls['c0w'])               # [128, 3, 4, 4]
    for dy in range(4):
        for dx in range(4):
            t = dy * 4 + dx
            wb[0:3, CLS_C0 + t * 128:CLS_C0 + (t + 1) * 128] = \
                c0[:, :, dy, dx].T
    bb[:128, 0] = _np(cls['c0b'])
    for i in range(3):
        wb[:, CLS_C1 + i * 128:CLS_C1 + (i + 1) * 128] = \
            _np(cls[f'c{i+1}w'])[:, :, 0, 0].T
        bb[:128, 1 + i] = _np(cls[f'c{i+1}b'])
    wb[:, CLS_C4:CLS_C4 + 32] = _np(cls['c4w'])[:, :, 0, 0].T
    bb[:32, 4] = _np(cls['c4b'])
    wb[0:32, CLS_FW:CLS_FW + 3] = _np(cls['fw']).T
    wb[32, CLS_FW:CLS_FW + 3] = _np(cls['fb'])
    return wb, bb


# =================== device program helpers ================================

class Resident:
    """Full-resident padded tensor: padded row r lives at tile row r."""

    def __init__(self, t):
        self.t = t

    def ap(self, pr0, rcount, col0, ccount, rstep=1, cstep=1, P=None):
        t = self.t
        p = t.shape[0] if P is None else P
        return t[0:p, pr0:pr0 + (rcount - 1) * rstep + 1:rstep,
                 col0:col0 + (ccount - 1) * cstep + 1:cstep]


class Ring:
    """Strip ring over a padded-row-indexed virtual tensor."""

    def __init__(self, nc, pool, tag, P, row_elems, dtype):
        self.nc, self.pool, self.tag = nc, pool, tag
        self.P, self.row_elems, self.dtype = P, row_elems, dtype
        self.rows = {}          # padded_row -> (tile, tile_row)
        self.cur = None
        self.lo = None

    def new_strip(self, lo, hi, gen_lo, gen_hi, zero_rows=(), wout=None):
        nc = self.nc
        win = hi - lo
        t = self.pool.tile([self.P, win, self.row_elems], self.dtype,
                           tag=self.tag)
        for pr in range(lo, gen_lo):
            if pr in zero_rows:
                continue
            src_t, src_r = self.rows[pr]
            nc.vector.tensor_copy(t[0:self.P, pr - lo, :],
                                  src_t[0:self.P, src_r, :])
        for pr in zero_rows:
            memset_(nc, t[0:self.P, pr - lo, :], 0.0)
        if gen_hi > gen_lo and wout is not None:
            # zero border columns of generated rows
            memset_(nc, t[0:self.P, gen_lo - lo:gen_hi - lo, 0], 0.0)
            memset_(nc, t[0:self.P, gen_lo - lo:gen_hi - lo,
                        wout + 1:self.row_elems], 0.0)
        self.rows = {pr: v for pr, v in self.rows.items() if v[0] is not t}
        for pr in range(lo, hi):
            self.rows[pr] = (t, pr - lo)
        self.cur, self.lo = t, lo
        return t

    def ap(self, pr0, rcount, col0, ccount, rstep=1, cstep=1, P=None):
        t, r0 = self.rows[pr0]
        for i in range(rcount):
            t2, r2 = self.rows[pr0 + i * rstep]
            assert t2 is t and r2 == r0 + i * rstep, \
                f"ring rows not contiguous at {pr0}+{i}*{rstep}"
        p = self.P if P is None else P
        return t[0:p, r0:r0 + (rcount - 1) * rstep + 1:rstep,
                 col0:col0 + (ccount - 1) * cstep + 1:cstep]


def memset_(nc, ap, val):
    if ap.dtype == F32R:
        ap = ap.bitcast(F32)
    nc.vector.memset(ap, val)


def chunk_ranges(r0, r1, rc):
    out = []
    r = r0
    while r < r1:
        out.append((r, min(rc, r1 - r)))
        r += rc
    return out


def emit_conv(nc, psum_pool, ptag, wr, wcol, Kin, Mtot, src, out_rows, Wout,
              writer, rc_max):
    """3x3 pad-1 conv over out rows [r0,r1): per-tap matmul accumulation.

    src.ap(padded_row, rcount, col0, ccount, P=Kin) -> rhs AP [Kin, rc, Wout]
    writer(m, mw, psum_tile, r0, rc) consumes the accumulated chunk.
    """
    mtiles = math.ceil(Mtot / 128)
    for (r0, rc) in chunk_ranges(out_rows[0], out_rows[1], rc_max):
        for m in range(mtiles):
            mw = min(128, Mtot - m * 128)
            ps = psum_pool.tile([128, rc_max * Wout], F32, tag=ptag)
            for t, (dy, dx) in enumerate(TAPS):
                rhs = src.ap(r0 + dy, rc, dx, Wout, P=Kin)
                nc.tensor.matmul(ps[0:mw, 0:rc * Wout],
                                 wr[0:Kin, wcol(t, m):wcol(t, m) + mw],
                                 rhs, start=(t == 0), stop=(t == 8))
            writer(m, mw, ps, r0, rc)


def emit_conv_ct4(nc, psum_pool, ptag, wr, wcol, Kin, M, src, out_rows, Wout,
                  writer, rc_max):
    """M<=32 conv, 4 chunks col-tiled into one PSUM bank (concurrent on PE).

    writer(pbase, ps, r0, rc) evacuates one chunk from psum partitions
    [pbase, pbase+M).
    """
    # NOTE: f32r matmuls reject tile_position (invalid ISA), so the four
    # chunks run as plain sequential chains for now.
    for (r0, rc) in chunk_ranges(out_rows[0], out_rows[1], rc_max):
        ps = psum_pool.tile([128, rc_max * Wout], F32, tag=ptag)
        for t, (dy, dx) in enumerate(TAPS):
            rhs = src.ap(r0 + dy, rc, dx, Wout, P=Kin)
            nc.tensor.matmul(ps[0:M, 0:rc * Wout],
                             wr[0:Kin, wcol(t, 0):wcol(t, 0) + M],
                             rhs, start=(t == 0), stop=(t == 8))
        writer(0, ps, r0, rc)


# =================== phase 2: generic SR net program =======================

def build_phase2(spc=SPC):
    nc = bacc.Bacc(None)
    x4 = nc.dram_tensor("x4", [spc, 3, H, W], F32R, kind="ExternalInput")
    wb4 = nc.dram_tensor("wb4", [spc, 64, WCOLS], F32R, kind="ExternalInput")
    bb4 = nc.dram_tensor("bb4", [spc, 128, NBCOL], F32, kind="ExternalInput")
    deb4 = nc.dram_tensor("deb4", [spc, 3, H, W], F32, kind="ExternalOutput")
    out4 = nc.dram_tensor("out4", [spc, 3, 4 * H, 4 * W], F32,
                          kind="ExternalOutput")

    with tile.TileContext(nc) as tc:
        with tc.tile_pool(name="res1", bufs=1) as res1, \
             tc.tile_pool(name="rings", bufs=2) as rings, \
             tc.tile_pool(name="stage", bufs=2) as stage, \
             tc.tile_pool(name="ph", bufs=2, space="PSUM") as ph, \
             tc.tile_pool(name="p1", bufs=2, space="PSUM") as p1, \
             tc.tile_pool(name="p2", bufs=2, space="PSUM") as p2, \
             tc.tile_pool(name="p3", bufs=2, space="PSUM") as p3:
            pools = (res1, rings, stage, ph, p1, p2, p3)
            for s in range(spc):
                emit_sample(nc, s, x4, wb4, bb4, deb4, out4, pools)
    nc.finalize()
    return nc


def emit_sample(nc, s, x4, wb4, bb4, deb4, out4, pools):
    res1, rings, stage, ph, p1, p2, p3 = pools
    WP64 = W + 2
    WP128 = 2 * W + 2
    WP256 = 4 * W + 2

    # ---- load + round weights (in-place f32r rounding) ----
    wr = res1.tile([64, WCOLS], F32R, tag="wr")
    nc.sync.dma_start(out=wr, in_=wb4[s])
    nc.vector.tensor_copy(wr, wr)
    bb = res1.tile([128, NBCOL], F32, tag="bb")
    nc.sync.dma_start(out=bb, in_=bb4[s])

    def bias(col, mw):
        return bb[0:mw, col:col + 1]

    # ---- x padded (f32r) ----
    xp = res1.tile([3, WP64, WP64], F32R, tag="xp")
    memset_(nc, xp[:], 0.0)
    nc.sync.dma_start(out=xp[0:3, 1:H + 1, 1:W + 1], in_=x4[s])
    nc.vector.tensor_copy(xp, xp)
    xpad = Resident(xp)

    # ---- feat (resident, padded) ----
    ft = res1.tile([64, WP64, WP64], F32R, tag="feat")
    memset_(nc, ft[0:64, 0, :], 0.0)
    memset_(nc, ft[0:64, H + 1, :], 0.0)
    memset_(nc, ft[0:64, 1:H + 1, 0], 0.0)
    memset_(nc, ft[0:64, 1:H + 1, W + 1], 0.0)
    feat = Resident(ft)

    def feat_prelu_writer(bias_ap):
        def wfn(m, mw, ps, r0, rc):
            nc.scalar.activation(ft[0:mw, r0 + 1:r0 + 1 + rc, 1:W + 1],
                                 ps[0:mw, 0:rc * W], PRELU,
                                 bias=bias_ap, alpha=ALPHA)
        return wfn

    def ring_prelu_writer(ring, bias_ap, wout):
        def wfn(m, mw, ps, r0, rc):
            tgt = ring.ap(r0 + 1, rc, 1, wout, P=mw)
            nc.scalar.activation(tgt, ps[0:mw, 0:rc * wout], PRELU,
                                 bias=bias_ap, alpha=ALPHA)
        return wfn

    # ---- cf conv: x -> feat ----
    emit_conv(nc, ph, "ph", wr, wcol_cf, 3, 64, xpad, (0, H), W,
              feat_prelu_writer(bias(BC_CF, 64)), 8)

    # ---- residual blocks ----
    for b in range(NB):
        tmpr = Ring(nc, rings, "tmp", 64, WP64, F32R)
        w1col = wcol_blk(2 * b)
        w2col = wcol_blk(2 * b + 1)
        b2ap = bias(BC_BLK + 2 * b + 1, 64)

        def c2_writer(m, mw, ps, r0, rc, _b2ap=b2ap):
            nc.vector.scalar_tensor_tensor(
                ft[0:64, r0 + 1:r0 + 1 + rc, 1:W + 1],
                ps[0:64, 0:rc * W], _b2ap,
                ft[0:64, r0 + 1:r0 + 1 + rc, 1:W + 1], ADD, ADD)

        for t in range(8):
            glo, ghi = 8 * t + 1, 8 * t + 9
            lo = max(0, 8 * t - 2)
            hi = ghi + (1 if t == 7 else 0)
            zr = [0] if t == 0 else ([H + 1] if t == 7 else [])
            tmpr.new_strip(lo, hi, glo, ghi, zero_rows=zr, wout=W)
            emit_conv(nc, ph, "ph", wr, w1col, 64, 64, feat,
                      (8 * t, 8 * t + 8), W,
                      ring_prelu_writer(tmpr, bias(BC_BLK + 2 * b, 64), W), 8)
            c0, c1 = (0, 6) if t == 0 else (8 * t - 2, 8 * t + 6)
            emit_conv(nc, ph, "ph", wr, w2col, 64, 64, tmpr, (c0, c1), W,
                      c2_writer, 8)
        emit_conv(nc, ph, "ph", wr, w2col, 64, 64, tmpr, (62, 64), W,
                  c2_writer, 8)

    # ---- deblur ----
    dstg = stage.tile([3, H, W], F32, tag="deb")

    def deb_writer(pbase, ps, r0, rc):
        nc.vector.scalar_tensor_tensor(
            dstg[0:3, r0:r0 + rc, :], ps[pbase:pbase + 3, 0:rc * W],
            bias(BC_DEB, 3),
            xp[0:3, r0 + 1:r0 + 1 + rc, 1:W + 1], ADD, ADD)

    emit_conv_ct4(nc, ph, "ph", wr, wcol_deb, 64, 3, feat, (0, H), W,
                  deb_writer, 8)
    nc.sync.dma_start(out=deb4[s], in_=dstg)

    # ---- tail: up1 -> h1 ring -> up2 -> h2 ring -> last -> out ----
    h1r = Ring(nc, rings, "h1", 64, WP128, F32R)
    h2r = Ring(nc, rings, "h2", 64, WP256, F32R)

    def shuffle_writer(ring, bias_base, wout):
        def wfn(m, mw, ps, r0, rc):
            for g in range(2):
                ab = 2 * m + g
                a, bsub = ab >> 1, ab & 1
                tgt = ring.ap(2 * r0 + a + 1, rc, 1 + bsub, wout,
                              rstep=2, cstep=2)
                nc.scalar.activation(
                    tgt, ps[g * 64:g * 64 + 64, 0:rc * wout], PRELU,
                    bias=bb[g * 64:g * 64 + 64,
                            bias_base + m:bias_base + m + 1],
                    alpha=ALPHA)
        return wfn

    up1_w = wcol_up(UP1_OFF)
    up2_w = wcol_up(UP2_OFF)

    def up1_strip(si):
        glo, ghi = 16 * si + 1, 16 * si + 17
        lo = max(0, 16 * si - 2)
        hi = ghi + (1 if si == 7 else 0)
        zr = [0] if si == 0 else ([2 * H + 1] if si == 7 else [])
        h1r.new_strip(lo, hi, glo, ghi, zero_rows=zr, wout=2 * W)
        emit_conv(nc, p1, "p1", wr, up1_w, 64, 256, feat,
                  (8 * si, 8 * si + 8), W,
                  shuffle_writer(h1r, BC_UP1, W), 8)

    def u2_strip(v):
        c0, c1 = max(0, 8 * v - 2), min(2 * H, 8 * v + 6)
        if c0 >= c1:
            return
        glo, ghi = 2 * c0 + 1, 2 * c1 + 1
        lo = max(0, glo - 3)
        hi = ghi + (1 if c1 == 2 * H else 0)
        zr = [0] if c0 == 0 else []
        if c1 == 2 * H:
            zr = zr + [4 * H + 1]
        h2r.new_strip(lo, hi, glo, ghi, zero_rows=zr, wout=4 * W)
        emit_conv(nc, p2, "p2", wr, up2_w, 64, 256, h1r, (c0, c1), 2 * W,
                  shuffle_writer(h2r, BC_UP2, 2 * W), 4)

    def last_strip(wi):
        d0, d1 = max(0, 16 * wi - 6), min(4 * H, 16 * wi + 10)
        for (g0, g1) in [(d0, min(d0 + 8, d1)), (min(d0 + 8, d1), d1)]:
            if g0 >= g1:
                continue
            ost = stage.tile([3, 8, 4 * W], F32, tag="ost")

            def last_writer(pbase, ps, r0, rc, _ost=ost, _g0=g0):
                nc.scalar.activation(
                    _ost[0:3, r0 - _g0:r0 - _g0 + rc, :],
                    ps[pbase:pbase + 3, 0:rc * 4 * W], IDENT,
                    bias=bias(BC_LAST, 3))

            emit_conv_ct4(nc, p3, "p3", wr, wcol_last, 64, 3, h2r, (g0, g1),
                          4 * W, last_writer, 2)
            nc.sync.dma_start(out=out4[s, :, g0:g1, :],
                              in_=ost[0:3, 0:g1 - g0, :])

    for si in range(9):
        if si < 8:
            up1_strip(si)
        for v in ([2 * si, 2 * si + 1] if si < 8 else [16]):
            u2_strip(v)
            last_strip(v)


# =================== phase 1: classifier program ===========================

def build_phase1(spc=SPC):
    nc = bacc.Bacc(None)
    x4 = nc.dram_tensor("x4", [spc, 3, H, W], F32R, kind="ExternalInput")
    cw = nc.dram_tensor("cw", [128, CLS_COLS], F32R, kind="ExternalInput")
    cb = nc.dram_tensor("cb", [128, 5], F32, kind="ExternalInput")
    lg4 = nc.dram_tensor("lg4", [spc, 3], F32, kind="ExternalOutput")

    with tile.TileContext(nc) as tc:
        with tc.tile_pool(name="sb", bufs=1) as sb, \
             tc.tile_pool(name="work", bufs=2) as work, \
             tc.tile_pool(name="pp", bufs=2, space="PSUM") as pp:
            wrt = sb.tile([128, CLS_COLS], F32R, tag="cw")
            nc.sync.dma_start(out=wrt, in_=cw[:, :])
            nc.vector.tensor_copy(wrt, wrt)
            cbt = sb.tile([128, 5], F32, tag="cb")
            nc.sync.dma_start(out=cbt, in_=cb[:, :])
            rhs = sb.tile([33, 4], F32R, tag="rhs")
            memset_(nc, rhs[32:33, :], 1.0)
            means = sb.tile([32, 4], F32, tag="means")

            for s in range(spc):
                xt = work.tile([3, H, W], F32R, tag="xt")
                nc.sync.dma_start(out=xt, in_=x4[s])
                nc.vector.tensor_copy(xt, xt)

                # c0: 4x4 stride-4 conv as 16 accumulated taps
                ps = pp.tile([128, 256], F32, tag="pp")
                for dy in range(4):
                    for dx in range(4):
                        t = dy * 4 + dx
                        rhs0 = xt[0:3, dy:dy + 61:4, dx:dx + 61:4]
                        nc.tensor.matmul(
                            ps, wrt[0:3, CLS_C0 + t * 128:
                                    CLS_C0 + (t + 1) * 128],
                            rhs0, start=(t == 0), stop=(t == 15))
                f = work.tile([128, 256], F32R, tag="f0")
                nc.scalar.activation(f, ps, PRELU, bias=cbt[0:128, 0:1],
                                     alpha=ALPHA)
                for i in range(3):
                    ps = pp.tile([128, 256], F32, tag="pp")
                    nc.tensor.matmul(ps, wrt[0:128, CLS_C1 + i * 128:
                                             CLS_C1 + (i + 1) * 128],
                                     f, start=True, stop=True)
                    f2 = work.tile([128, 256], F32R, tag="f0")
                    nc.scalar.activation(f2, ps, PRELU,
                                         bias=cbt[0:128, 1 + i:2 + i],
                                         alpha=ALPHA)
                    f = f2
                ps = pp.tile([128, 256], F32, tag="pp")
                nc.tensor.matmul(ps[0:32, :], wrt[0:128, CLS_C4:CLS_C4 + 32],
                                 f, start=True, stop=True)
                dummy = work.tile([32, 256], F32, tag="dummy")
                nc.scalar.activation(dummy, ps[0:32, :], COPY,
                                     scale=1.0 / 256.0,
                                     accum_out=means[0:32, s:s + 1])
            nc.vector.tensor_scalar_add(rhs[0:32, :], means, cbt[0:32, 4:5])
            psl = pp.tile([3, 4], F32, tag="ppl")
            nc.tensor.matmul(psl, wrt[0:33, CLS_FW:CLS_FW + 3], rhs,
                             start=True, stop=True)
            lg = sb.tile([3, 4], F32, tag="lg")
            nc.vector.tensor_copy(lg, psl)
            base = lg4[:, :]
            dstap = bass.AP(tensor=base.tensor, offset=base.offset,
                            ap=[[1, 3], [3, spc]])
            nc.sync.dma_start(out=dstap, in_=lg[0:3, 0:spc])
    nc.finalize()
    return nc


# =================== orchestration ========================================

_CACHE = {}
TRACE = False           # set True (e.g. from test.py) to profile HW time
LAST_EXEC_NS = []       # per-phase exec_time_ns when TRACE


def _get_programs():
    if "p1" not in _CACHE:
        _CACHE["p1"] = build_phase1()
    if "p2" not in _CACHE:
        _CACHE["p2"] = build_phase2()
    return _CACHE["p1"], _CACHE["p2"]


def kernel(x, params):
    x = np.ascontiguousarray(np.asarray(x, dtype=np.float32))
    nc1, nc2 = _get_programs()

    LAST_EXEC_NS.clear()
    cw, cb = pack_classifier(params['cls'])
    in_maps = [{"x4": x[c * SPC:(c + 1) * SPC], "cw": cw, "cb": cb}
               for c in range(N_CORES)]
    r1 = run_bass_kernel_spmd(nc1, in_maps, core_ids=list(range(N_CORES)),
                              trace=TRACE)
    if TRACE:
        LAST_EXEC_NS.append(r1.exec_time_ns)
    logits = np.concatenate([r1.results[c]["lg4"] for c in range(N_CORES)])
    flags = np.argmax(logits, axis=1)

    tmpl = [pack_generic_net(params[k], nf, nb)
            for k, nf, nb in (('net1', 64, 3), ('net2', 52, 2),
                              ('net3', 36, 1))]
    in_maps = []
    for c in range(N_CORES):
        wb = np.stack([tmpl[flags[c * SPC + i]][0] for i in range(SPC)])
        bbm = np.stack([tmpl[flags[c * SPC + i]][1] for i in range(SPC)])
        in_maps.append({"x4": x[c * SPC:(c + 1) * SPC], "wb4": wb,
                        "bb4": bbm})
    r2 = run_bass_kernel_spmd(nc2, in_maps, core_ids=list(range(N_CORES)),
                              trace=TRACE)
    if TRACE:
        LAST_EXEC_NS.append(r2.exec_time_ns)
    deblur = np.concatenate([r2.results[c]["deb4"] for c in range(N_CORES)])
    out = np.concatenate([r2.results[c]["out4"] for c in range(N_CORES)])
    return deblur, out, logits
